# revision 62
# baseline (speedup 1.0000x reference)
"""ChebNet (K=2) GNN message passing on 8 Trainium2 NeuronCores.

Sharding: nodes (x rows / dst segments) across 8 cores; weights replicated.
The edge weight w_e = -dis[src]*dis[dst] is folded into the endpoints:
the x table rows are dis-prescaled ON HOST, accumulators are scaled by
-dis on the dst side, so no per-edge weight is materialized.

Per layer, per core: dma_gather (256B rows, 4 source windows for the int16
index range) -> one strided tensor_reduce per call (degree-sorted uniform
slot ladder, dst groups on fixed SBUF partitions) -> dma_scatter_add into an
HBM accumulator. Feature tables are built via AllGather of per-core shards.
Dense phases (x@W0 + acc@W1, bias, relu, log_softmax) use PE transposes +
matmuls with DVE/ACT epilogues.

v2: deep gather pipeline — gathers issue up to SDEPTH calls ahead of the
reduce/scatter chain so all 4 SWDGE queues stay fed (the per-queue ring
drain rate ~4.4ns/packet is the floor; baseline had <=1 queue busy).
"""

import numpy as np

N = 100000
E = 1600000
DIN, HID, DOUT = 64, 32, 40
NC = 8                 # cores
S = 12544              # shard rows (= 98*128)
SPAD = 128             # pad rows per shard
SS = S + SPAD          # shard stride in table = 12672
NT = NC * SS           # table rows = 101376
NW = 4                 # src windows
WS = NT // NW          # window size = 25344
ZIDX = S               # window-local zero-row idx
NCOLS = S // 128       # 98
MAXIDX = 3072          # max gather idxs per call
MAXW = 10              # max dst-ranks per call
NSTRIP = 14            # dense-phase strip width (98 = 7*14)
NGT = 8                # gather tile buffers
NRT = 4                # reduce tile buffers
SDEPTH = 7             # scatter trails gather issue by this many calls


# ------------------------------------------------------------- host prep
def _build_structures(src, dst):
    deg = np.bincount(dst, minlength=N).astype(np.int64)

    core = dst // S
    d_loc = dst - core * S
    p = d_loc % 128
    col = d_loc // 128

    s_src = src // S
    trow = s_src * SS + (src - s_src * S)
    q = trow // WS
    lidx = (trow - q * WS).astype(np.int16)

    key = (core * NW + q) * S + d_loc
    cqdeg = np.bincount(key, minlength=NC * NW * S).reshape(NC, NW, S)
    cq = cqdeg.reshape(NC, NW, NCOLS, 128).transpose(0, 1, 3, 2)

    order = np.argsort(-cq, axis=-1, kind="stable")
    dsort = np.take_along_axis(cq, order, axis=-1)

    ladder = dsort.max(axis=(0, 1, 2))
    assert (np.diff(ladder) <= 0).all()

    calls = []
    r = 0
    while r < NCOLS and ladder[r] > 0:
        r2 = r
        while r2 < NCOLS and ladder[r2] == ladder[r]:
            r2 += 1
        khat = int(ladder[r])
        wmax = min(MAXW, max(1, MAXIDX // (128 * khat)))
        rr = r
        while rr < r2:
            w = min(wmax, r2 - rr)
            calls.append((khat, w, rr))
            rr += w
        r = r2
    rplus = r

    base = np.zeros(NCOLS + 1, np.int64)
    base[1:] = np.cumsum(ladder)
    sigma = int(base[rplus])

    rank_of = np.empty_like(order)
    np.put_along_axis(rank_of, order, np.arange(NCOLS)[None, None, None, :],
                      axis=-1)
    erank = rank_of[core, q, p, col]

    ekey = ((core * NW + q) * 128 + p) * NCOLS + erank
    eord = np.argsort(ekey, kind="stable")
    ks = ekey[eord]
    grp_start = np.r_[0, np.flatnonzero(np.diff(ks)) + 1]
    cum = np.arange(E) - np.repeat(grp_start, np.diff(np.r_[grp_start, E]))
    j = np.empty(E, np.int64)
    j[eord] = cum

    f = (base[erank] + j) * 128 + p
    gvals = np.full((NC, NW, sigma * 128), ZIDX, np.int16)
    gvals[core, q, f] = lidx

    dloc_sorted = order * 128 + np.arange(128)[None, None, :, None]

    def wrap(a):
        return a.reshape(-1, 16).T.copy()

    gblob, sblob = [], []
    for c in range(NC):
        gsegs, ssegs = [], []
        for qq in range(NW):
            off = 0
            for khat, w, r0 in calls:
                nidx = 128 * w * khat
                gsegs.append(wrap(gvals[c, qq, off:off + nidx]))
                off += nidx
                sv = dloc_sorted[c, qq, :, r0:r0 + w].T.reshape(-1) \
                    .astype(np.int16)
                ssegs.append(wrap(sv))
            assert off == sigma * 128
        gblob.append(np.tile(np.concatenate(gsegs, axis=1), (8, 1)))
        sblob.append(np.tile(np.concatenate(ssegs, axis=1), (8, 1)))

    return deg, calls, sigma, rplus, gblob, sblob


# ------------------------------------------------------------- program
def _build_program(calls, sigma, rplus):
    import contextlib
    import concourse.bacc as bacc
    import concourse.bass as bass
    import concourse.mybir as mybir
    from concourse import library_config

    f32 = mybir.dt.float32
    i16 = mybir.dt.int16
    AX = mybir.AxisListType
    OP = mybir.AluOpType
    AF = mybir.ActivationFunctionType

    GW = NW * sigma * 8
    SW = NW * rplus * 8
    TC = len(calls)
    TL = NW * TC
    call_nb = [w * k for k, w, _ in calls]
    NBMAX = max(call_nb)
    assert NBMAX <= MAXIDX // 128

    nc = bacc.Bacc("TRN2", num_swdge_queues=4)

    x_shard = nc.dram_tensor("x_shard", [SS, 64], f32, kind="ExternalInput")
    xT = nc.dram_tensor("xT", [65, S], f32, kind="ExternalInput")
    dis_in = nc.dram_tensor("dis_in", [128, 99], f32, kind="ExternalInput")
    ndis_in = nc.dram_tensor("ndis_in", [128, 99], f32, kind="ExternalInput")
    gblob = nc.dram_tensor("gblob", [128, GW], i16, kind="ExternalInput")
    sblob = nc.dram_tensor("sblob", [128, SW], i16, kind="ExternalInput")
    w1 = nc.dram_tensor("w1", [65, 32], f32, kind="ExternalInput")
    w1b = nc.dram_tensor("w1b", [64, 32], f32, kind="ExternalInput")
    w2 = nc.dram_tensor("w2", [33, 40], f32, kind="ExternalInput")
    w2b = nc.dram_tensor("w2b", [32, 40], f32, kind="ExternalInput")

    out_t = nc.dram_tensor("out", [S, DOUT], f32, kind="ExternalOutput")
    accum1 = nc.dram_tensor("accum1", [S, 64], f32, kind="ExternalOutput")
    accum2 = nc.dram_tensor("accum2", [S, 64], f32, kind="ExternalOutput")

    ag_in = nc.dram_tensor("ag_in", [SS, 64], f32)
    xs_full = nc.dram_tensor("xs_full", [NT, 64], f32, addr_space="Shared")
    htab = nc.dram_tensor("htab", [SS, 64], f32)
    hT_dr = nc.dram_tensor("hT_dr", [32, S], f32)
    ht_full = nc.dram_tensor("ht_full", [NT, 64], f32, addr_space="Shared")

    ctx = contextlib.ExitStack()
    sbuf = lambda name, shape, dt=f32: ctx.enter_context(
        nc.sbuf_tensor(name, shape, dt))
    psum = lambda name: ctx.enter_context(
        nc.psum_tensor(name, [128, 512], f32))
    sem = lambda name: ctx.enter_context(nc.semaphore(name))

    # ---- static per-call metadata and scatter bookkeeping (shared by the
    # gpsimd and vector emitters; python-level, compile-time only)
    meta = []
    for qq in range(NW):
        for ci, (khat, w, r0) in enumerate(calls):
            goff = qq * sigma * 8 + sum(call_nb[:ci]) * 8
            soff = qq * rplus * 8 + r0 * 8
            meta.append((qq, khat, w, goff, soff, 128 * w * khat))

    def nsc(c, s):
        """#scatters with global call index < c on sem slot s."""
        return (c - s + 3) // 4 if c > s else 0

    with ctx:
        gt = [sbuf(f"gt{i}", [128, NBMAX, 64]) for i in range(NGT)]
        rt = [sbuf(f"rt{i}", [128, MAXW, 64]) for i in range(NRT)]
        rt2 = [sbuf(f"rt2{i}", [128, MAXW, 32]) for i in range(NRT)]
        gb_sb = sbuf("gb_sb", [128, GW], i16)
        sb_sb = sbuf("sb_sb", [128, SW], i16)
        astrip = [sbuf(f"astrip{i}", [128, NSTRIP, 64]) for i in range(2)]
        a2strip = [sbuf(f"a2strip{i}", [128, NSTRIP, 32]) for i in range(2)]
        xTs = [sbuf(f"xTs{i}", [65, NSTRIP * 128]) for i in range(2)]
        accT = [sbuf(f"accT{i}", [64, 128]) for i in range(2)]
        a2T = [sbuf(f"a2T{i}", [32, 128]) for i in range(2)]
        hTw_sb = [sbuf(f"hTw_sb{i}", [32, NSTRIP * 128]) for i in range(2)]
        hTl_sb = [sbuf(f"hTl_sb{i}", [33, NSTRIP * 128]) for i in range(2)]
        ht_sb = sbuf("ht_sb", [128, NCOLS, 32])
        h_tile = [sbuf(f"h_tile{i}", [128, 32]) for i in range(2)]
        o_sb = sbuf("o_sb", [128, NCOLS, 40])
        dis_sb = sbuf("dis_sb", [128, 99])
        ndis_sb = sbuf("ndis_sb", [128, 99])
        w1_sb = sbuf("w1_sb", [65, 32])
        w1b_sb = sbuf("w1b_sb", [64, 32])
        w2_sb = sbuf("w2_sb", [33, 40])
        w2b_sb = sbuf("w2b_sb", [32, 40])
        ident = sbuf("ident", [128, 128])
        zrow = sbuf("zrow", [128, 64])
        esc = [sbuf(f"esc{i}", [128, 40]) for i in range(2)]
        nmx_sb = sbuf("nmx_sb", [128, NCOLS])
        ssum_sb = sbuf("ssum_sb", [128, NCOLS])
        ls_sb = sbuf("ls_sb", [128, NCOLS])
        m2_sb = sbuf("m2_sb", [128, NCOLS])

        ptA, ptB = psum("ptA"), psum("ptB")      # full banks
        pmA, pmB = psum("pmA"), psum("pmB")

        in_w = sem("in_w")
        agp = sem("agp")
        zm = sem("zm")
        idsem = sem("idsem")
        agx = sem("agx")
        zf = sem("zf")
        gsemQ = [sem(f"gsemq{i}") for i in range(4)]
        rsem = sem("rsem")
        ssemQ = [sem(f"ssemq{i}") for i in range(NRT)]
        agh = sem("agh")
        ardS = [sem("ard0"), sem("ard1")]
        nas = sem("nas")
        xTlS = [sem("xTl0"), sem("xTl1")]
        tp = sem("tp")
        tpev = sem("tpev")
        mms = sem("mms")
        hev = sem("hev")
        htp = sem("htp")
        htev = sem("htev")
        htw = sem("htw")
        hTwsS = [sem("hTws0"), sem("hTws1")]
        hTlsS = [sem("hTls0"), sem("hTls1")]
        a2rdS = [sem("a2rd0"), sem("a2rd1")]
        na2 = sem("na2")
        tp2 = sem("tp2")
        tp2ev = sem("tp2ev")
        mm2s = sem("mm2s")
        obs = sem("obs")
        obsc = sem("obsc")
        htse = sem("htse")
        onesd = sem("onesd")
        acte = sem("acte")
        find = sem("find")
        ow = sem("ow")

        with nc.Block() as block:

            # ---------------- sync: loads & stores ------------------
            @block.sync
            def _(sync: bass.BassEngine):
                sync.dma_start(ag_in[:], x_shard[:]).then_inc(agp, 16)
                for t_sb, t_dr in ((gb_sb, gblob), (sb_sb, sblob),
                                   (dis_sb, dis_in), (ndis_sb, ndis_in),
                                   (w1_sb, w1), (w1b_sb, w1b), (w2_sb, w2),
                                   (w2b_sb, w2b)):
                    sync.dma_start(t_sb[:], t_dr[:]).then_inc(in_w, 16)
                # htab zero-fill (zrow memset by gpsimd -> zm)
                sync.wait_ge(zm, 2)
                hv = htab[:].rearrange("(b p) d -> p b d", p=128)
                sync.dma_start(
                    hv[:, :, :],
                    zrow[:].unsqueeze(1).to_broadcast([128, 99, 64]),
                ).then_inc(zf, 16)

                # P4 strips
                a1v = accum1[:].rearrange("(g p) d -> p g d", p=128)
                for s_ in range(NRT):
                    sync.wait_ge(ssemQ[s_], 16 * nsc(TL, s_))
                for st in range(7):
                    sl = slice(st * NSTRIP, (st + 1) * NSTRIP)
                    if st >= 2:
                        sync.wait_ge(tp, NSTRIP * (st - 1))
                        sync.wait_ge(mms, NSTRIP * (st - 1))
                    sync.dma_start(astrip[st % 2][:, :, :], a1v[:, sl, :]
                                   ).then_inc(ardS[st % 2], 16)
                    sync.dma_start(
                        xTs[st % 2][:, :],
                        xT[:, st * NSTRIP * 128:(st + 1) * NSTRIP * 128]
                    ).then_inc(xTlS[st % 2], 16)
                    # hT strip writeback to DRAM (one strip behind)
                    if st >= 1:
                        sync.wait_ge(htev, NSTRIP * st)
                        sync.dma_start(
                            hT_dr[:, (st - 1) * NSTRIP * 128:
                                  st * NSTRIP * 128],
                            hTw_sb[(st - 1) % 2][:, :]
                        ).then_inc(hTwsS[(st - 1) % 2], 16)
                sync.wait_ge(htev, NSTRIP * 7)
                sync.dma_start(
                    hT_dr[:, 6 * NSTRIP * 128:7 * NSTRIP * 128],
                    hTw_sb[6 % 2][:, :]).then_inc(hTwsS[0], 16)
                # ht table write (htev >= NCOLS implies all DVE hts done)
                sync.wait_ge(htse, 1)
                sync.wait_ge(zf, 16)
                sync.dma_start(hv[:, :NCOLS, 0:32], ht_sb[:, :, :]
                               ).then_inc(htw, 16)

                # P7 strips
                a2v = accum2[:].rearrange("(g p) d -> p g d", p=128)
                for s_ in range(NRT):
                    sync.wait_ge(ssemQ[s_], 16 * nsc(2 * TL, s_))
                for st in range(7):
                    sl = slice(st * NSTRIP, (st + 1) * NSTRIP)
                    if st >= 2:
                        sync.wait_ge(tp2, NSTRIP * (st - 1))
                    sync.dma_start(a2strip[st % 2][:, :, :],
                                   a2v[:, sl, 0:32]).then_inc(a2rdS[st % 2], 16)
                    if st >= 2:
                        sync.wait_ge(mm2s, NSTRIP * (st - 1))
                    sync.dma_start(
                        hTl_sb[st % 2][0:32, :],
                        hT_dr[:, st * NSTRIP * 128:(st + 1) * NSTRIP * 128]
                    ).then_inc(hTlsS[st % 2], 16)
                # final out
                ov = out_t[:].rearrange("(g p) d -> p g d", p=128)
                sync.wait_ge(find, 1)
                sync.dma_start(ov[:, :, :], o_sb[:, :, :]).then_inc(ow, 16)
                sync.wait_ge(ow, 16)

            # ---------------- gpsimd ---------------------------------
            @block.gpsimd
            def _(g: bass.BassGpSimd):
                g.memset(ident[:], 0.0).then_inc(zm, 1)
                g.memset(zrow[:], 0.0).then_inc(zm, 1)
                g.wait_ge(zm, 2)
                g.affine_select(
                    out=ident[:], in_=ident[:],
                    compare_op=OP.not_equal, fill=1.0, base=0,
                    pattern=[[-1, 128]], channel_multiplier=1,
                ).then_inc(idsem, 1)
                g.load_library(library_config.mlp)
                # x table AllGather: x_shard is host-prescaled; only a
                # DRAM->DRAM bounce (collectives can't read IO tensors).
                g.wait_ge(agp, 16)
                g.collective_compute(
                    "AllGather", OP.bypass,
                    ins=[ag_in[:]], outs=[xs_full[:]],
                    replica_groups=[list(range(NC))],
                ).then_inc(agx, 1)
                g.wait_ge(in_w, 16 * 8)
                g.wait_ge(agx, 1)

                def emit_layer(layer):
                    tbase = layer * TL

                    def scatter(tl):
                        # serialize across window boundaries: same dst rows
                        # are RMW-scattered once per window; concurrent CCE
                        # read-modify-writes on the same address must not
                        # overlap in the SDMA pipeline.
                        c = tbase + tl
                        if tl % TC == 0 and tl > 0:
                            for s_ in range(NRT):
                                g.wait_ge(ssemQ[s_], 16 * nsc(c, s_))
                        qq, khat, w, goff, soff, nidx = meta[tl]
                        if layer == 0:
                            g.dma_scatter_add(
                                accum1[:, :], rt[c % NRT][:, :w, :],
                                sb_sb[:, soff:soff + 8 * w],
                                128 * w, 128 * w, 64,
                                single_packet=False, queue_num=c % 4,
                            ).then_inc(ssemQ[c % NRT], 16)
                        else:
                            g.dma_scatter_add(
                                accum2[:, 0:32], rt2[c % NRT][:, :w, :],
                                sb_sb[:, soff:soff + 8 * w],
                                128 * w, 128 * w, 32, elem_step=64,
                                single_packet=False, queue_num=c % 4,
                            ).then_inc(ssemQ[c % NRT], 16)

                    for tl in range(TL):
                        qq, khat, w, goff, soff, nidx = meta[tl]
                        t = tbase + tl
                        if tl >= NGT:
                            g.wait_ge(rsem, t - NGT + 1)
                        g.dma_gather(
                            gt[t % NGT][:, :w * khat, :],
                            tbl[qq * WS:(qq + 1) * WS, :],
                            gb_sb[:, goff:goff + nidx // 16],
                            nidx, nidx, 64, single_packet=False,
                            queue_num=t % 4,
                        ).then_inc(gsemQ[t % 4], 16)
                        if tl >= SDEPTH:
                            g.wait_ge(rsem, t - SDEPTH + 1)
                            scatter(tl - SDEPTH)
                    for tl in range(TL - SDEPTH, TL):
                        g.wait_ge(rsem, tbase + tl + 1)
                        scatter(tl)

                tbl = xs_full
                emit_layer(0)
                g.wait_ge(htw, 16)
                g.collective_compute(
                    "AllGather", OP.bypass,
                    ins=[htab[:]], outs=[ht_full[:]],
                    replica_groups=[list(range(NC))],
                ).then_inc(agh, 1)
                g.wait_ge(agh, 1)
                tbl = ht_full
                emit_layer(1)

            # ---------------- vector ---------------------------------
            @block.vector
            def _(v):
                v.wait_ge(in_w, 16 * 8)
                # ones rows of hTl slots (row 32), set once
                v.memset(hTl_sb[0][32:33, :], 1.0)
                v.memset(hTl_sb[1][32:33, :], 1.0)
                v.drain().then_inc(onesd, 1)

                def reduces(layer):
                    tbase = layer * TL
                    for tl in range(TL):
                        qq, khat, w, goff, soff, nidx = meta[tl]
                        t = tbase + tl
                        v.wait_ge(gsemQ[t % 4], 16 * (t // 4 + 1))
                        if t >= NRT:
                            v.wait_ge(ssemQ[t % NRT],
                                      16 * (nsc(t - NRT, t % NRT) + 1))
                        src = gt[t % NGT][:, :w * khat, :].rearrange(
                            "p (j k) d -> p j d k", k=khat)
                        if layer == 0:
                            v.tensor_reduce(
                                out=rt[t % NRT][:, :w, :], in_=src,
                                axis=AX.X, op=OP.add).then_inc(rsem, 1)
                        else:
                            v.tensor_reduce(
                                out=rt2[t % NRT][:, :w, :],
                                in_=src[:, :, 0:32, :],
                                axis=AX.X, op=OP.add).then_inc(rsem, 1)

                reduces(0)

                for g_ in range(NCOLS):
                    st, jj = divmod(g_, NSTRIP)
                    s2 = st % 2
                    slt = g_ % 2
                    pt = ptA if slt == 0 else ptB
                    pm = pmA if slt == 0 else pmB
                    if jj == 0:
                        v.wait_ge(ardS[st % 2], 16 * (st // 2 + 1))
                        v.tensor_tensor(
                            out=astrip[s2][:, :, :],
                            in0=astrip[s2][:, :, :],
                            in1=ndis_sb[:, st * NSTRIP:(st + 1) * NSTRIP]
                                .unsqueeze(2).to_broadcast(
                                    [128, NSTRIP, 64]),
                            op=OP.mult).then_inc(nas, 1)
                    v.wait_ge(tp, g_ + 1)
                    if g_ >= 2:
                        v.wait_ge(mms, g_ - 1)
                    v.tensor_copy(out=accT[slt][:, :], in_=pt[0:64, 0:128]
                                  ).then_inc(tpev, 1)
                    v.wait_ge(mms, g_ + 1)
                    if g_ >= 2:
                        v.wait_ge(htp, g_ - 1)
                    v.tensor_scalar(out=ht_sb[:, g_, :], in0=pm[:, 0:32],
                                    scalar1=0.0,
                                    scalar2=dis_sb[:, g_:g_ + 1],
                                    op0=OP.max, op1=OP.mult)
                    v.tensor_scalar_max(out=h_tile[slt][:, :],
                                        in0=pm[:, 0:32], scalar1=0.0
                                        ).then_inc(hev, 1)
                    v.wait_ge(htp, g_ + 1)
                    if jj == 0 and st >= 2:
                        v.wait_ge(hTwsS[st % 2], 16 * ((st - 2) // 2 + 1))
                    v.tensor_copy(
                        out=hTw_sb[s2][:, jj * 128:(jj + 1) * 128],
                        in_=pt[0:32, 0:128]).then_inc(htev, 1)

                v.drain().then_inc(htse, 1)

                reduces(1)

                for g_ in range(NCOLS):
                    st, jj = divmod(g_, NSTRIP)
                    s2 = st % 2
                    slt = g_ % 2
                    pt = ptA if slt == 0 else ptB
                    pm = pmA if slt == 0 else pmB
                    if jj == 0:
                        v.wait_ge(a2rdS[st % 2], 16 * (st // 2 + 1))
                        v.tensor_tensor(
                            out=a2strip[s2][:, :, :],
                            in0=a2strip[s2][:, :, :],
                            in1=ndis_sb[:, st * NSTRIP:(st + 1) * NSTRIP]
                                .unsqueeze(2).to_broadcast(
                                    [128, NSTRIP, 32]),
                            op=OP.mult).then_inc(na2, 1)
                    v.wait_ge(tp2, g_ + 1)
                    if g_ >= 2:
                        v.wait_ge(mm2s, g_ - 1)
                    v.tensor_copy(out=a2T[slt][:, :], in_=pt[0:32, 0:128]
                                  ).then_inc(tp2ev, 1)
                    v.wait_ge(mm2s, g_ + 1)
                    v.tensor_copy(out=o_sb[:, g_, :], in_=pm[:, 0:40]
                                  ).then_inc(obsc, 1)
                    v.tensor_reduce(out=nmx_sb[:, g_:g_ + 1],
                                    in_=pm[:, 0:40],
                                    axis=AX.X, op=OP.max, negate=True
                                    ).then_inc(obs, 1)
                v.wait_ge(acte, NCOLS + 1)
                v.tensor_tensor(out=m2_sb[:, :], in0=nmx_sb[:, :],
                                in1=ls_sb[:, :], op=OP.subtract)
                v.drain()
                v.tensor_tensor(
                    out=o_sb[:, :, :], in0=o_sb[:, :, :],
                    in1=m2_sb[:].unsqueeze(2).to_broadcast(
                        [128, NCOLS, 40]),
                    op=OP.add).then_inc(find, 1)

            # ---------------- scalar (ACT) ---------------------------
            @block.scalar
            def _(a):
                for g_ in range(NCOLS):
                    a.wait_ge(obs, g_ + 1)
                    a.wait_ge(obsc, g_ + 1)
                    if g_ >= 2:
                        a.wait_ge(acte, g_ - 1)
                    a.activation(out=esc[g_ % 2][:, :], in_=o_sb[:, g_, :],
                                 func=AF.Exp,
                                 bias=nmx_sb[:, g_:g_ + 1],
                                 accum_out=ssum_sb[:, g_:g_ + 1]
                                 ).then_inc(acte, 1)
                a.drain()
                a.activation(out=ls_sb[:, :], in_=ssum_sb[:, :], func=AF.Ln
                             ).then_inc(acte, 1)

            # ---------------- tensor (PE) ----------------------------
            @block.tensor
            def _(te):
                te.wait_ge(in_w, 16 * 8)
                te.wait_ge(idsem, 1)
                for g_ in range(NCOLS):
                    st, jj = divmod(g_, NSTRIP)
                    s2 = st % 2
                    slt = g_ % 2
                    pt = ptA if slt == 0 else ptB
                    pm = pmA if slt == 0 else pmB
                    te.wait_ge(nas, st + 1)
                    if g_ >= 2:
                        te.wait_ge(htev, g_ - 1)
                    te.transpose(out=pt[0:64, 0:128],
                                 in_=astrip[s2][:, jj, :],
                                 identity=ident[:]).then_inc(tp, 1)
                    te.wait_ge(tpev, g_ + 1)
                    te.wait_ge(xTlS[st % 2], 16 * (st // 2 + 1))
                    if g_ >= 2:
                        te.wait_ge(hev, g_ - 1)
                    te.matmul(out=pm[:, 0:32],
                              lhsT=xTs[s2][0:65, jj * 128:(jj + 1) * 128],
                              rhs=w1_sb[:, :], start=True, stop=False)
                    te.matmul(out=pm[:, 0:32], lhsT=accT[slt][:, :],
                              rhs=w1b_sb[:, :], start=False, stop=True
                              ).then_inc(mms, 1)
                    te.wait_ge(hev, g_ + 1)
                    te.transpose(out=pt[0:32, 0:128],
                                 in_=h_tile[slt][:, :],
                                 identity=ident[:]).then_inc(htp, 1)
                te.wait_ge(htev, NCOLS)
                te.wait_ge(hev, NCOLS)
                te.wait_ge(onesd, 1)
                for g_ in range(NCOLS):
                    st, jj = divmod(g_, NSTRIP)
                    s2 = st % 2
                    slt = g_ % 2
                    pt = ptA if slt == 0 else ptB
                    pm = pmA if slt == 0 else pmB
                    te.wait_ge(na2, st + 1)
                    if g_ >= 2:
                        te.wait_ge(tp2ev, g_ - 1)
                    te.transpose(out=pt[0:32, 0:128],
                                 in_=a2strip[s2][:, jj, :],
                                 identity=ident[:]).then_inc(tp2, 1)
                    te.wait_ge(tp2ev, g_ + 1)
                    if g_ >= 2:
                        te.wait_ge(obs, g_ - 1)
                        te.wait_ge(obsc, g_ - 1)
                    te.wait_ge(hTlsS[st % 2], 16 * (st // 2 + 1))
                    te.matmul(out=pm[:, 0:40],
                              lhsT=hTl_sb[s2][0:33, jj * 128:(jj + 1) * 128],
                              rhs=w2_sb[:, :], start=True, stop=False)
                    te.matmul(out=pm[:, 0:40], lhsT=a2T[slt][:, :],
                              rhs=w2b_sb[:, :], start=False, stop=True
                              ).then_inc(mm2s, 1)

    nc.compile()
    return nc


_prog_cache = {}
TRACE = False
LAST_RES = None


def kernel(**inputs):
    global LAST_RES
    from concourse.bass_utils import run_bass_kernel_spmd

    x = np.asarray(inputs["x"], dtype=np.float32)
    ei = np.asarray(inputs["edge_index"])
    src = ei[0].astype(np.int64)
    dst = ei[1].astype(np.int64)
    W0_1 = np.asarray(inputs["W0_1"], np.float32)
    W1_1 = np.asarray(inputs["W1_1"], np.float32)
    b1 = np.asarray(inputs["b1"], np.float32)
    W0_2 = np.asarray(inputs["W0_2"], np.float32)
    W1_2 = np.asarray(inputs["W1_2"], np.float32)
    b2 = np.asarray(inputs["b2"], np.float32)

    deg, calls, sigma, rplus, gblob, sblob = _build_structures(src, dst)

    key = (tuple(calls), sigma, rplus)
    if key not in _prog_cache:
        _prog_cache[key] = _build_program(calls, sigma, rplus)
    nc = _prog_cache[key]

    deg_f = deg.astype(np.float32)
    dis = np.where(deg_f > 0,
                   (1.0 / np.sqrt(np.maximum(deg_f, 1.0).astype(np.float32))
                    ).astype(np.float32),
                   np.float32(0.0)).astype(np.float32)

    x_pad = np.zeros((NC * S, DIN), np.float32)
    x_pad[:N] = x
    dis_pad = np.zeros(NC * S, np.float32)
    dis_pad[:N] = dis

    in_maps = []
    for c in range(NC):
        sl = slice(c * S, (c + 1) * S)
        xs = np.zeros((SS, 64), np.float32)
        xs[:S] = x_pad[sl] * dis_pad[sl, None]
        xTp = np.ones((65, S), np.float32)
        xTp[:64] = x_pad[sl].T
        dl = dis_pad[sl].reshape(NCOLS, 128).T
        dl99 = np.zeros((128, 99), np.float32)
        dl99[:, :NCOLS] = dl
        in_maps.append({
            "x_shard": xs,
            "xT": xTp,
            "dis_in": dl99,
            "ndis_in": -dl99,
            "gblob": gblob[c],
            "sblob": sblob[c],
            "w1": np.concatenate([W0_1, b1[None, :]], 0),
            "w1b": W1_1,
            "w2": np.concatenate([W0_2, b2[None, :]], 0),
            "w2b": W1_2,
        })

    res = run_bass_kernel_spmd(nc, in_maps, list(range(NC)), trace=TRACE)
    LAST_RES = res
    if res.exec_time_ns is not None:
        print(f"HW exec time: {res.exec_time_ns} ns")
    out = np.concatenate([res.results[c]["out"] for c in range(NC)], axis=0)
    return out[:N].astype(np.float32)
